# revision 26
# baseline (speedup 1.0000x reference)
"""Bass/Tile kernel for nn_EncoderHead: single-head encoder attention.

Per-core (data-parallel over batch B=8 across 8 NeuronCores):
  x_b [T=2048, C=768], Wq/Wk/Wv [C, H=64], mask_b [1, T] (0 = masked key)
  out_b [T, H] = softmax((x Wq)(x Wk)^T * C**-0.5, masked) @ (x Wv)

Layout strategy (all on-chip after the initial loads):
  - xT [C, T] built by PE transposes of x tiles.
  - qT, kT [H, T] = [Wq|Wk]^T @ xT in one packed matmul chain (contraction
    over c on partitions, 128 stationary columns).
  - vT [H, T] likewise, then PE-transposed to V' [T, H+1] where
    V'[s, 0:H] = v[s,:] * mask[s] and V'[s, H] = mask[s].
  - S^T[s, t] = sum_h kT[h,s] qT[h,t]   (s on partitions, t free).
  - P^T = exp(scale * S^T)  -- no max subtraction needed: logits are O(1)
    (softmax is shift-invariant; reference only shifts for stability).
  - outT'[h', t] = sum_s V'[s, h'] P^T[s, t]  accumulated over s-chunks in
    PSUM; row H is the masked softmax denominator (ones-column trick).
  - transpose outT' back, divide by denominator per-partition, DMA out.

Scheduling: emission order is software-pipelined (Tile keeps per-engine FIFO
order): x-transposes of group g+1 are emitted before projections of group g,
and S^T+exp of s-chunk js+1 before the PV matmuls of js, so the PE stream
stays dense and ACT (exp) runs back-to-back.
"""

import os
import sys

import numpy as np

_TRN_REPO = "/opt/trn_rl_repo"
if _TRN_REPO not in sys.path and os.path.isdir(_TRN_REPO):
    sys.path.insert(0, _TRN_REPO)

B, T, C, H = 8, 2048, 768, 64
P = 128  # partitions
NT = T // P      # 16 t-chunks of 128
NC = C // P      # 6 c-chunks of 128
NS = T // P      # 16 s-chunks of 128
FT = 512         # matmul moving free-dim tile
NFT = T // FT    # 4 free-dim tiles
SCALE = float(C) ** -0.5

_CACHE = {}


def _build():
    from contextlib import ExitStack

    import concourse.bass as bass  # noqa: F401
    import concourse.tile as tile
    from concourse import bacc, mybir
    from concourse.masks import make_identity

    f32 = mybir.dt.float32
    f32r = mybir.dt.float32r
    i32 = mybir.dt.int32
    EXP = mybir.ActivationFunctionType.Exp

    nc = bacc.Bacc(
        "TRN2",
        target_bir_lowering=False,
        debug=False,
        enable_asserts=False,
        num_devices=8,
    )

    x = nc.dram_tensor("x", [T, C], f32r, kind="ExternalInput").ap()
    mask = nc.dram_tensor("mask", [1, T], i32, kind="ExternalInput").ap()
    wq = nc.dram_tensor("Wq", [C, H], f32r, kind="ExternalInput").ap()
    wk = nc.dram_tensor("Wk", [C, H], f32r, kind="ExternalInput").ap()
    wv = nc.dram_tensor("Wv", [C, H], f32r, kind="ExternalInput").ap()
    out = nc.dram_tensor("out", [T, H], f32, kind="ExternalOutput").ap()

    with tile.TileContext(nc) as tc, ExitStack() as ctx:
        const = ctx.enter_context(tc.tile_pool(name="const", bufs=1))

        ident = const.tile([P, P], f32)
        make_identity(nc, ident)
        ident_r = const.tile([P, P], f32r)
        nc.gpsimd.tensor_copy(ident_r, ident)

        # Packed [Wq | Wk] stationary chunks: wqk_sb[:, j, 0:H] = Wq chunk j,
        # [:, j, H:2H] = Wk chunk j. One matmul chain produces qT and kT.
        wqk_sb = const.tile([P, NC, 2 * H], f32r)
        nc.scalar.dma_start(
            out=wqk_sb[:, :, 0:H], in_=wq.rearrange("(n p) h -> p n h", p=P)
        )
        nc.scalar.dma_start(
            out=wqk_sb[:, :, H : 2 * H],
            in_=wk.rearrange("(n p) h -> p n h", p=P),
        )
        wv_sb = const.tile([P, NC, H], f32r)
        nc.scalar.dma_start(
            out=wv_sb, in_=wv.rearrange("(n p) h -> p n h", p=P)
        )

        # mask as per-partition column per s-chunk: msk_f[p, n] = mask[n*P+p].
        # Load [16,128] natural (contiguous rows), cast, then one PE transpose.
        msk_i = const.tile([NS, P], i32)
        nc.scalar.dma_start(
            out=msk_i, in_=mask.rearrange("a (n p) -> (a n) p", p=P)
        )
        msk_n = const.tile([NS, P], f32)
        nc.vector.tensor_copy(msk_n, msk_i)
        msk_f = const.tile([P, NS], f32)

        xT_sb = const.tile([P, NC, T], f32r)      # 48KB/partition
        qT_sb = const.tile([H, T], f32r)
        kT_sb = const.tile([H, T], f32r)
        vp_sb = const.tile([P, NS, H + 1], f32r)  # V' chunks

        # ---- Phase 1+2: load x, transpose, project, build V' ----
        with ExitStack() as p12:
            xin = p12.enter_context(tc.tile_pool(name="xin", bufs=6))
            pst = p12.enter_context(
                tc.tile_pool(name="pst", bufs=5, space="PSUM")
            )
            psp = p12.enter_context(
                tc.tile_pool(name="psp", bufs=2, space="PSUM")
            )
            psv = p12.enter_context(
                tc.tile_pool(name="psv", bufs=1, space="PSUM")
            )
            vT_sb = p12.enter_context(tc.tile_pool(name="vts", bufs=1)).tile(
                [H, T], f32
            )

            def emit_mask():
                pm = psv.tile([P, H], f32, tag="pv", name="pm")
                nc.tensor.transpose(pm[:, 0:NS], msk_n, ident[0:NS, 0:NS])
                nc.vector.tensor_copy(msk_f, pm[:, 0:NS])

            def emit_chunk(it):
                x_tile = xin.tile([P, C], f32r, name="x_tile")
                nc.sync.dma_start(
                    out=x_tile, in_=x[it * P : (it + 1) * P, :]
                )
                # 6 transposed [128,128] blocks packed into 2 PSUM banks,
                # then 2 strided copies into the xT c-planes.
                for gg, nblk in ((0, 4), (4, 2)):
                    pt = pst.tile([P, 512], f32r, tag="pt", name="pt")
                    for b in range(nblk):
                        jc = gg + b
                        nc.tensor.transpose(
                            pt[:, b * P : (b + 1) * P],
                            x_tile[:, jc * P : (jc + 1) * P],
                            ident_r,
                        )
                    nc.any.tensor_copy(
                        xT_sb[:, gg : gg + nblk, it * P : (it + 1) * P],
                        pt[:, 0 : nblk * P].rearrange(
                            "p (c t) -> p c t", c=nblk
                        ),
                    )

            qk_chain = {}
            v_chain = {}

            def emit_qk_chain(tt, j0, j1):
                # q,k packed: psum rows 0:H = qT tile, H:2H = kT tile
                if tt not in qk_chain:
                    qk_chain[tt] = psp.tile([P, FT], f32, tag="pp", name="pp")
                pp = qk_chain[tt]
                for jc in range(j0, j1):
                    nc.tensor.matmul(
                        pp,
                        lhsT=wqk_sb[:, jc, :],
                        rhs=xT_sb[:, jc, tt * FT : (tt + 1) * FT],
                        start=(jc == 0),
                        stop=(jc == NC - 1),
                    )
                if j1 == NC:
                    nc.any.tensor_copy(
                        qT_sb[:, tt * FT : (tt + 1) * FT], pp[0:H, :]
                    )
                    nc.any.tensor_copy(
                        kT_sb[:, tt * FT : (tt + 1) * FT], pp[H : 2 * H, :]
                    )

            def emit_v_chain(tt, j0, j1):
                if tt not in v_chain:
                    v_chain[tt] = psp.tile([H, FT], f32, tag="pp", name="pv_ps")
                pv_ps = v_chain[tt]
                for jc in range(j0, j1):
                    nc.tensor.matmul(
                        pv_ps,
                        lhsT=wv_sb[:, jc, :],
                        rhs=xT_sb[:, jc, tt * FT : (tt + 1) * FT],
                        start=(jc == 0),
                        stop=(jc == NC - 1),
                    )
                if j1 == NC:
                    nc.any.tensor_copy(
                        vT_sb[:, tt * FT : (tt + 1) * FT], pv_ps
                    )

            def emit_vp(tt, k):
                # V' chunk: v rows * mask, plus the mask ones-column
                # (masked softmax denominator trick).
                js = 4 * tt + k
                pv = psv.tile([P, H], f32, tag="pv", name="pv")
                nc.tensor.transpose(
                    pv, vT_sb[:, js * P : (js + 1) * P], ident[0:H, 0:H]
                )
                nc.vector.tensor_scalar_mul(
                    vp_sb[:, js, 0:H], pv, msk_f[:, js : js + 1]
                )
                nc.vector.tensor_copy(
                    vp_sb[:, js, H : H + 1], msk_f[:, js : js + 1]
                )

            # software pipeline: per-chunk interleave, projections one
            # t-tile group behind the transposes
            proj_parts = {}

            def proj_ops(tt):
                ops = []
                ops.append(lambda: emit_qk_chain(tt, 0, 3))
                ops.append(lambda: emit_qk_chain(tt, 3, 6))
                ops.append(lambda: emit_v_chain(tt, 0, 3))
                ops.append(lambda: emit_v_chain(tt, 3, 6))
                for k in range(4):
                    ops.append(lambda k=k: emit_vp(tt, k))
                return ops

            for it in range(NT + 4):
                if it < NT:
                    emit_chunk(it)
                if it == 1:
                    emit_mask()
                tt = it // 4 - 1
                if tt >= 0:
                    if tt not in proj_parts:
                        proj_parts[tt] = proj_ops(tt)
                    k = it % 4
                    ops = proj_parts[tt]
                    take = ops[:2] if k < 3 else ops[:]
                    del ops[: len(take)]
                    for op in take:
                        op()

        # ---- Phase 3: S^T -> exp -> accumulate outT', per t-half ----
        # Two passes over t (1024 each) so out_ps takes 2 PSUM banks and the
        # S^T tiles triple-buffer (6 banks). Work is emitted in 3-js bursts
        # (6 S^T matmuls, then 6 PV matmuls of the previous burst) so the PE
        # sees long uninterrupted matmul runs and stays at full clock.
        TH = 2 * FT  # 1024
        outT_sb = ctx.enter_context(tc.tile_pool(name="outts", bufs=1)).tile(
            [H + 1, T], f32
        )
        with ExitStack() as p3:
            pso = p3.enter_context(
                tc.tile_pool(name="pso", bufs=1, space="PSUM")
            )
            pss = p3.enter_context(
                tc.tile_pool(name="pss", bufs=3, space="PSUM")
            )
            pexp = p3.enter_context(tc.tile_pool(name="pexp", bufs=6))
            from concourse.tile import add_dep_helper

            GROUPS = [list(range(s, min(s + 3, NS))) for s in range(0, NS, 3)]

            for h in range(2):
                out_ps = pso.tile([H + 1, TH], f32, tag="outps", name="outps")
                pe_tiles = {}
                st_last = {}

                def st_group(grp):
                    for js in grp:
                        pe = pexp.tile([P, TH], f32r, tag="pe", name="pe")
                        pe_tiles[js] = pe
                        ps = pss.tile([P, TH], f32, tag="ps", name="ps")
                        for u in range(2):
                            tt = 2 * h + u
                            mm = nc.tensor.matmul(
                                ps[:, u * FT : (u + 1) * FT],
                                lhsT=kT_sb[:, js * P : (js + 1) * P],
                                rhs=qT_sb[:, tt * FT : (tt + 1) * FT],
                                start=True,
                                stop=True,
                            )
                        nc.scalar.activation(pe, ps, EXP, scale=SCALE)
                        st_last[js] = mm

                def pv_group(grp, gate):
                    for js in grp:
                        pe = pe_tiles.pop(js)
                        for u in range(2):
                            mm = nc.tensor.matmul(
                                out_ps[:, u * FT : (u + 1) * FT],
                                lhsT=vp_sb[:, js, :],
                                rhs=pe[:, u * FT : (u + 1) * FT],
                                start=(js == 0),
                                stop=(js == NS - 1),
                            )
                            if gate is not None:
                                add_dep_helper(
                                    mm.ins, gate.ins, sync=False,
                                    reason="phase3 burst order",
                                )

                st_group(GROUPS[0])
                for gi, grp in enumerate(GROUPS):
                    gate = None
                    if gi + 1 < len(GROUPS) and len(GROUPS[gi + 1]) > 1:
                        nxt = GROUPS[gi + 1]
                        # first STs of next group reuse slots already freed;
                        # the last ST waits on this group's last exp, so emit
                        # it behind the ready PV matmuls
                        st_group(nxt[:-1])
                        pv_group(grp[:-1], st_last[nxt[-2]])
                        st_group(nxt[-1:])
                        pv_group(grp[-1:], st_last[nxt[-1]])
                    elif gi + 1 < len(GROUPS):
                        st_group(GROUPS[gi + 1])
                        pv_group(grp, st_last[GROUPS[gi + 1][-1]])
                    else:
                        pv_group(grp, gate)
                for u in range(2):
                    nc.vector.tensor_copy(
                        outT_sb[:, h * TH + u * FT : h * TH + (u + 1) * FT],
                        out_ps[:, u * FT : (u + 1) * FT],
                    )

        # ---- Phase 4: normalize + transpose back + store ----
        with ExitStack() as p4:
            psf = p4.enter_context(
                tc.tile_pool(name="psf", bufs=4, space="PSUM")
            )
            fin = p4.enter_context(tc.tile_pool(name="fin", bufs=4))
            oall = p4.enter_context(tc.tile_pool(name="oall", bufs=1)).tile(
                [P, NT, H], f32
            )
            for it in range(NT):
                pf = psf.tile([P, H + 1], f32, tag="pf", name="pf")
                nc.tensor.transpose(
                    pf,
                    outT_sb[:, it * P : (it + 1) * P],
                    ident[0 : H + 1, 0 : H + 1],
                )
                rec = fin.tile([P, 1], f32, tag="rec", name="rec")
                nc.vector.reciprocal(rec, pf[:, H : H + 1])
                nc.scalar.mul(oall[:, it, :], pf[:, 0:H], rec)
            nc.sync.dma_start(
                out=out.rearrange("(n p) h -> p n h", p=P), in_=oall
            )

    nc.compile()
    return nc


def _get_nc():
    if "nc" not in _CACHE:
        _CACHE["nc"] = _build()
    return _CACHE["nc"]


def kernel(**inputs) -> np.ndarray:
    from concourse import bass_utils

    x = np.ascontiguousarray(np.asarray(inputs["x"], dtype=np.float32))
    mask = np.ascontiguousarray(np.asarray(inputs["mask"], dtype=np.int32))
    wq = np.ascontiguousarray(np.asarray(inputs["Wq"], dtype=np.float32))
    wk = np.ascontiguousarray(np.asarray(inputs["Wk"], dtype=np.float32))
    wv = np.ascontiguousarray(np.asarray(inputs["Wv"], dtype=np.float32))

    nc = _get_nc()
    in_maps = [
        {"x": x[b], "mask": mask[b], "Wq": wq, "Wk": wk, "Wv": wv}
        for b in range(B)
    ]
    last_err = None
    for _attempt in range(3):
        try:
            res = bass_utils.run_bass_kernel_spmd(
                nc, in_maps, core_ids=list(range(B)),
                **_CACHE.get("run_kwargs", {}),
            )
            break
        except Exception as e:  # transient NRT device errors: retry
            last_err = e
    else:
        raise last_err
    _CACHE["last_results"] = res
    return np.stack([res.results[b]["out"] for b in range(B)], axis=0)


# revision 27
# speedup vs baseline: 1.2926x; 1.2926x over previous
"""Bass/Tile kernel for nn_EncoderHead: single-head encoder attention.

Per-core (data-parallel over batch B=8 across 8 NeuronCores):
  x_b [T=2048, C=768], Wq/Wk/Wv [C, H=64], mask_b [1, T] (0 = masked key)
  out_b [T, H] = softmax((x Wq)(x Wk)^T * C**-0.5, masked) @ (x Wv)

Layout strategy (all on-chip after the initial loads):
  - xT [C, T] built by PE transposes of x tiles.
  - qT, kT [H, T] = [Wq|Wk]^T @ xT in one packed matmul chain (contraction
    over c on partitions, 128 stationary columns).
  - vT [H, T] likewise, then PE-transposed to V' [T, H+1] where
    V'[s, 0:H] = v[s,:] * mask[s] and V'[s, H] = mask[s].
  - S^T[s, t] = sum_h kT[h,s] qT[h,t]   (s on partitions, t free).
  - P^T = exp(scale * S^T)  -- no max subtraction needed: logits are O(1)
    (softmax is shift-invariant; reference only shifts for stability).
  - outT'[h', t] = sum_s V'[s, h'] P^T[s, t]  accumulated over s-chunks in
    PSUM; row H is the masked softmax denominator (ones-column trick).
  - transpose outT' back, divide by denominator per-partition, DMA out.

Scheduling: emission order is software-pipelined (Tile keeps per-engine FIFO
order): x-transposes of group g+1 are emitted before projections of group g,
and S^T+exp of s-chunk js+1 before the PV matmuls of js, so the PE stream
stays dense and ACT (exp) runs back-to-back.
"""

import os
import sys

import numpy as np

_TRN_REPO = "/opt/trn_rl_repo"
if _TRN_REPO not in sys.path and os.path.isdir(_TRN_REPO):
    sys.path.insert(0, _TRN_REPO)

B, T, C, H = 8, 2048, 768, 64
P = 128  # partitions
NT = T // P      # 16 t-chunks of 128
NC = C // P      # 6 c-chunks of 128
NS = T // P      # 16 s-chunks of 128
FT = 512         # matmul moving free-dim tile
NFT = T // FT    # 4 free-dim tiles
SCALE = float(C) ** -0.5

_CACHE = {}


def _build():
    from contextlib import ExitStack

    import concourse.bass as bass  # noqa: F401
    import concourse.tile as tile
    from concourse import bacc, mybir
    from concourse.masks import make_identity

    f32 = mybir.dt.float32
    f32r = mybir.dt.float32r
    i32 = mybir.dt.int32
    EXP = mybir.ActivationFunctionType.Exp

    nc = bacc.Bacc(
        "TRN2",
        target_bir_lowering=False,
        debug=False,
        enable_asserts=False,
        num_devices=8,
    )

    x = nc.dram_tensor("x", [T, C], f32r, kind="ExternalInput").ap()
    mask = nc.dram_tensor("mask", [1, T], i32, kind="ExternalInput").ap()
    wq = nc.dram_tensor("Wq", [C, H], f32r, kind="ExternalInput").ap()
    wk = nc.dram_tensor("Wk", [C, H], f32r, kind="ExternalInput").ap()
    wv = nc.dram_tensor("Wv", [C, H], f32r, kind="ExternalInput").ap()
    out = nc.dram_tensor("out", [T, H], f32, kind="ExternalOutput").ap()

    with tile.TileContext(nc) as tc, ExitStack() as ctx:
        const = ctx.enter_context(tc.tile_pool(name="const", bufs=1))

        ident = const.tile([P, P], f32)
        make_identity(nc, ident)
        ident_r = const.tile([P, P], f32r)
        nc.gpsimd.tensor_copy(ident_r, ident)

        # Packed [Wq | Wk] stationary chunks: wqk_sb[:, j, 0:H] = Wq chunk j,
        # [:, j, H:2H] = Wk chunk j. One matmul chain produces qT and kT.
        wqk_sb = const.tile([P, NC, 2 * H], f32r)
        nc.scalar.dma_start(
            out=wqk_sb[:, :, 0:H], in_=wq.rearrange("(n p) h -> p n h", p=P)
        )
        nc.scalar.dma_start(
            out=wqk_sb[:, :, H : 2 * H],
            in_=wk.rearrange("(n p) h -> p n h", p=P),
        )
        wv_sb = const.tile([P, NC, H], f32r)
        nc.scalar.dma_start(
            out=wv_sb, in_=wv.rearrange("(n p) h -> p n h", p=P)
        )

        # mask as per-partition column per s-chunk: msk_f[p, n] = mask[n*P+p].
        # Load [16,128] natural (contiguous rows), cast, then one PE transpose.
        msk_i = const.tile([NS, P], i32)
        nc.scalar.dma_start(
            out=msk_i, in_=mask.rearrange("a (n p) -> (a n) p", p=P)
        )
        msk_n = const.tile([NS, P], f32)
        nc.vector.tensor_copy(msk_n, msk_i)
        msk_f = const.tile([P, NS], f32)

        xT_sb = const.tile([P, NC, T], f32r)      # 48KB/partition
        qT_sb = const.tile([H, T], f32r)
        kT_sb = const.tile([H, T], f32r)
        vp_sb = const.tile([P, NS, H + 1], f32r)  # V' chunks

        # ---- Phase 1+2: load x, transpose, project, build V' ----
        with ExitStack() as p12:
            xin = p12.enter_context(tc.tile_pool(name="xin", bufs=4))
            pst = p12.enter_context(
                tc.tile_pool(name="pst", bufs=4, space="PSUM")
            )
            psp = p12.enter_context(
                tc.tile_pool(name="psp", bufs=2, space="PSUM")
            )
            psv = p12.enter_context(
                tc.tile_pool(name="psv", bufs=2, space="PSUM")
            )
            vT_sb = p12.enter_context(tc.tile_pool(name="vts", bufs=1)).tile(
                [H, T], f32
            )

            def emit_mask():
                pm = psv.tile([P, H], f32, tag="pv", name="pm")
                nc.tensor.transpose(pm[:, 0:NS], msk_n, ident[0:NS, 0:NS])
                nc.vector.tensor_copy(msk_f, pm[:, 0:NS])

            def emit_chunk(it):
                x_tile = xin.tile([P, C], f32r, name="x_tile")
                nc.sync.dma_start(
                    out=x_tile, in_=x[it * P : (it + 1) * P, :]
                )
                # 6 transposed [128,128] blocks packed into 2 PSUM banks,
                # then 2 strided copies into the xT c-planes.
                for gg, nblk in ((0, 4), (4, 2)):
                    pt = pst.tile([P, 512], f32r, tag="pt", name="pt")
                    for b in range(nblk):
                        jc = gg + b
                        nc.tensor.transpose(
                            pt[:, b * P : (b + 1) * P],
                            x_tile[:, jc * P : (jc + 1) * P],
                            ident_r,
                        )
                    nc.any.tensor_copy(
                        xT_sb[:, gg : gg + nblk, it * P : (it + 1) * P],
                        pt[:, 0 : nblk * P].rearrange(
                            "p (c t) -> p c t", c=nblk
                        ),
                    )

            qk_chain = {}
            v_chain = {}

            def emit_qk_chain(tt, j0, j1):
                # q,k packed: psum rows 0:H = qT tile, H:2H = kT tile
                if tt not in qk_chain:
                    qk_chain[tt] = psp.tile([P, FT], f32, tag="pp", name="pp")
                pp = qk_chain[tt]
                for jc in range(j0, j1):
                    nc.tensor.matmul(
                        pp,
                        lhsT=wqk_sb[:, jc, :],
                        rhs=xT_sb[:, jc, tt * FT : (tt + 1) * FT],
                        start=(jc == 0),
                        stop=(jc == NC - 1),
                    )
                if j1 == NC:
                    nc.any.tensor_copy(
                        qT_sb[:, tt * FT : (tt + 1) * FT], pp[0:H, :]
                    )
                    nc.any.tensor_copy(
                        kT_sb[:, tt * FT : (tt + 1) * FT], pp[H : 2 * H, :]
                    )

            def emit_v_chain(tt, j0, j1):
                if tt not in v_chain:
                    v_chain[tt] = psp.tile([H, FT], f32, tag="pp", name="pv_ps")
                pv_ps = v_chain[tt]
                for jc in range(j0, j1):
                    nc.tensor.matmul(
                        pv_ps,
                        lhsT=wv_sb[:, jc, :],
                        rhs=xT_sb[:, jc, tt * FT : (tt + 1) * FT],
                        start=(jc == 0),
                        stop=(jc == NC - 1),
                    )
                if j1 == NC:
                    nc.any.tensor_copy(
                        vT_sb[:, tt * FT : (tt + 1) * FT], pv_ps
                    )

            def emit_vp(tt, k):
                # V' chunk: v rows * mask, plus the mask ones-column
                # (masked softmax denominator trick).
                js = 4 * tt + k
                pv = psv.tile([P, H], f32, tag="pv", name="pv")
                nc.tensor.transpose(
                    pv, vT_sb[:, js * P : (js + 1) * P], ident[0:H, 0:H]
                )
                nc.vector.tensor_scalar_mul(
                    vp_sb[:, js, 0:H], pv, msk_f[:, js : js + 1]
                )
                nc.vector.tensor_copy(
                    vp_sb[:, js, H : H + 1], msk_f[:, js : js + 1]
                )

            # software pipeline: per-chunk interleave, projections one
            # t-tile group behind the transposes
            proj_parts = {}

            def proj_ops(tt):
                ops = []
                ops.append(lambda: emit_qk_chain(tt, 0, 3))
                ops.append(lambda: emit_qk_chain(tt, 3, 6))
                ops.append(lambda: emit_v_chain(tt, 0, 3))
                ops.append(lambda: emit_v_chain(tt, 3, 6))
                for k in range(4):
                    ops.append(lambda k=k: emit_vp(tt, k))
                return ops

            for it in range(NT + 4):
                if it < NT:
                    emit_chunk(it)
                if it == 1:
                    emit_mask()
                tt = it // 4 - 1
                if tt >= 0:
                    if tt not in proj_parts:
                        proj_parts[tt] = proj_ops(tt)
                    k = it % 4
                    ops = proj_parts[tt]
                    take = ops[:2] if k < 3 else ops[:]
                    del ops[: len(take)]
                    for op in take:
                        op()

        # ---- Phase 3: S^T -> exp -> accumulate outT', per t-half ----
        # Two passes over t (1024 each) so out_ps takes 2 PSUM banks and the
        # S^T tiles triple-buffer (6 banks). Work is emitted in 3-js bursts
        # (6 S^T matmuls, then 6 PV matmuls of the previous burst) so the PE
        # sees long uninterrupted matmul runs and stays at full clock.
        TH = 2 * FT  # 1024
        outT_sb = ctx.enter_context(tc.tile_pool(name="outts", bufs=1)).tile(
            [H + 1, T], f32
        )
        with ExitStack() as p3:
            pso = p3.enter_context(
                tc.tile_pool(name="pso", bufs=1, space="PSUM")
            )
            pss = p3.enter_context(
                tc.tile_pool(name="pss", bufs=3, space="PSUM")
            )
            pexp = p3.enter_context(tc.tile_pool(name="pexp", bufs=6))
            from concourse.tile import add_dep_helper

            GROUPS = [list(range(s, min(s + 3, NS))) for s in range(0, NS, 3)]

            for h in range(2):
                out_ps = pso.tile([H + 1, TH], f32, tag="outps", name="outps")
                pe_tiles = {}
                st_last = {}

                def st_group(grp):
                    for js in grp:
                        pe = pexp.tile([P, TH], f32r, tag="pe", name="pe")
                        pe_tiles[js] = pe
                        ps = pss.tile([P, TH], f32, tag="ps", name="ps")
                        for u in range(2):
                            tt = 2 * h + u
                            mm = nc.tensor.matmul(
                                ps[:, u * FT : (u + 1) * FT],
                                lhsT=kT_sb[:, js * P : (js + 1) * P],
                                rhs=qT_sb[:, tt * FT : (tt + 1) * FT],
                                start=True,
                                stop=True,
                            )
                        nc.scalar.activation(pe, ps, EXP, scale=SCALE)
                        st_last[js] = mm

                def pv_group(grp, gate):
                    for js in grp:
                        pe = pe_tiles.pop(js)
                        for u in range(2):
                            mm = nc.tensor.matmul(
                                out_ps[:, u * FT : (u + 1) * FT],
                                lhsT=vp_sb[:, js, :],
                                rhs=pe[:, u * FT : (u + 1) * FT],
                                start=(js == 0),
                                stop=(js == NS - 1),
                            )
                            if gate is not None:
                                add_dep_helper(
                                    mm.ins, gate.ins, sync=False,
                                    reason="phase3 burst order",
                                )

                st_group(GROUPS[0])
                for gi, grp in enumerate(GROUPS):
                    gate = None
                    if gi + 1 < len(GROUPS):
                        st_group(GROUPS[gi + 1])
                        gate = st_last[GROUPS[gi + 1][-1]]
                    pv_group(grp, gate)
                for u in range(2):
                    nc.vector.tensor_copy(
                        outT_sb[:, h * TH + u * FT : h * TH + (u + 1) * FT],
                        out_ps[:, u * FT : (u + 1) * FT],
                    )

        # ---- Phase 4: normalize + transpose back + store ----
        with ExitStack() as p4:
            psf = p4.enter_context(
                tc.tile_pool(name="psf", bufs=4, space="PSUM")
            )
            fin = p4.enter_context(tc.tile_pool(name="fin", bufs=4))
            oall = p4.enter_context(tc.tile_pool(name="oall", bufs=1)).tile(
                [P, NT, H], f32
            )
            for it in range(NT):
                pf = psf.tile([P, H + 1], f32, tag="pf", name="pf")
                nc.tensor.transpose(
                    pf,
                    outT_sb[:, it * P : (it + 1) * P],
                    ident[0 : H + 1, 0 : H + 1],
                )
                rec = fin.tile([P, 1], f32, tag="rec", name="rec")
                nc.vector.reciprocal(rec, pf[:, H : H + 1])
                nc.scalar.mul(oall[:, it, :], pf[:, 0:H], rec)
            nc.sync.dma_start(
                out=out.rearrange("(n p) h -> p n h", p=P), in_=oall
            )

    nc.compile()
    return nc


def _get_nc():
    if "nc" not in _CACHE:
        _CACHE["nc"] = _build()
    return _CACHE["nc"]


def kernel(**inputs) -> np.ndarray:
    from concourse import bass_utils

    x = np.ascontiguousarray(np.asarray(inputs["x"], dtype=np.float32))
    mask = np.ascontiguousarray(np.asarray(inputs["mask"], dtype=np.int32))
    wq = np.ascontiguousarray(np.asarray(inputs["Wq"], dtype=np.float32))
    wk = np.ascontiguousarray(np.asarray(inputs["Wk"], dtype=np.float32))
    wv = np.ascontiguousarray(np.asarray(inputs["Wv"], dtype=np.float32))

    nc = _get_nc()
    in_maps = [
        {"x": x[b], "mask": mask[b], "Wq": wq, "Wk": wk, "Wv": wv}
        for b in range(B)
    ]
    last_err = None
    for _attempt in range(3):
        try:
            res = bass_utils.run_bass_kernel_spmd(
                nc, in_maps, core_ids=list(range(B)),
                **_CACHE.get("run_kwargs", {}),
            )
            break
        except Exception as e:  # transient NRT device errors: retry
            last_err = e
    else:
        raise last_err
    _CACHE["last_results"] = res
    return np.stack([res.results[b]["out"] for b in range(B)], axis=0)
